# revision 43
# baseline (speedup 1.0000x reference)
"""Trainium2 Bass kernel for DampedAttention.

Full inputs in, full output out. Sharding: 8 cores = 2 batches x 4 head-groups
(4 heads of dim 64 each per core). Per core:

  QT/KT  [c, s] transposed projections (c on partitions), scale 1/8 and biases
         folded in (bias via K=1 ones-row matmuls, scale into weights on host)
  V      [s, c] natural projection (lhsT for the P@V matmul)
  ST     scores transposed [k, q] per (k-chunk, q-block) so exp(ST) is directly
         the lhsT-layout P^T needed by P@V -- no on-chip transposes
  ctxT   [65, q] = V_aug^T @ P^T ; row 64 = softmax row-sums (ones column in V)
  LVT    [64, q] banded 0.4*L^T matmuls (8 unique host-built band tiles)
  blend  ctxT_final = PV * (0.6/r, bcast over partitions) + LVT
  out    [s, o] natural out-projection; host sums 4 head-group partials + bo

v2 restructure (vs the 315us baseline):
  - exp/ln pinned to the natural_log_exp_and_others ACT table set (the
    baseline thrashed 17 ACT_TABLE_LOADs between exp_and_others/natural_log)
  - projections emitted dc-outer so the PE starts as soon as the first xt
    chunk lands (baseline idled 25us waiting for the full input DMA)
  - c-tile-1 Q/K projections are emitted interleaved into attention hp=0,
    and the out-projection per q-block interleaved into hp=1, so ScalarE
    (the critical engine) starts exp ~30us in and never drains
  - softmax epilogue emission deferred past the next q-block's first
    scores+exp so the ScalarE stream has no per-q-block bubble
  - 5 of 16 k-chunks compute exp on the DVE via the int16 bit-trick
    (bitcast bf16 exp), offloading the ScalarE bottleneck
  - output partials in bf16 (halves the output DMA)

Matmul operands are bf16; accumulation, softmax row-sums, reciprocal and the
0.6/r normalization stay fp32. The entropy gate in the reference is a forward
no-op and is skipped. Softmax max-subtraction is skipped (scores are O(1)).
"""
import numpy as np
import ml_dtypes

S = 2048
D = 1024
CLOC = 256          # channels per core (4 heads x 64)
HD = 64
NH = 4              # heads per core
NDC = 8             # 128-wide d-chunks in contraction D
NKC = 16            # 128-wide k/s chunks in S
NQB = 4             # 512-wide q blocks
QB = 512
WINDOW = 3
STRENGTH = 0.4
EPS = 1e-10
F32 = np.float32
BF16 = ml_dtypes.bfloat16
F8 = ml_dtypes.float8_e4m3
SW = 256.0   # fp8 weight pre-scale (keeps W out of fp8 subnormals);
             # 1/SW^2 is folded into the exp() input scale

# DVE bit-trick exp: exp(x) ~= bitcast_bf16(int16(x * 128/ln2 + 16256 - C))
# C tunes the sawtooth bias; +0.5 emulates round-to-nearest under truncation.
A_EXP = float(128.0 / np.log(2.0))
B_EXP = float(16256 - 8 + 0.5)
# alternate exp between DVE (evens) and ScalarE (odds): consecutive chunks'
# exps run on different engines concurrently, so the scores-psum recycling
# chain (bufs=2) never clocks the attention loop at one engine's exp latency
DVE_KCS = (0, 2, 4, 6, 8, 10, 12, 14)
SCORE_SCALE = 1.0 / (SW * SW)   # undo the fp8 weight pre-scale at exp time


def _build_L04T():
    i = np.arange(S)
    d = (i[:, None] - i[None, :]).astype(F32)
    k = np.where(np.abs(d) <= WINDOW,
                 np.exp(-(d ** 2) / F32(2.0 * STRENGTH ** 2)),
                 F32(0.0)).astype(F32)
    L = k / (k.sum(axis=-1, keepdims=True) + F32(EPS))
    return (F32(0.4) * L).T.copy()  # [s, q], pre-scaled by (1 - lambda_jump)


def _lt_tiles():
    """Unique [128, 512] band tiles of 0.4*L^T plus (qb -> [(j, uniq_idx)])."""
    L04T = _build_L04T()
    uniq = []
    slots = {qb: [] for qb in range(NQB)}
    for qb in range(NQB):
        for j in range(max(0, qb * 4 - 1), min(NKC, qb * 4 + 5)):
            t = L04T[j * 128:(j + 1) * 128, qb * QB:(qb + 1) * QB]
            for ui, ut in enumerate(uniq):
                if np.array_equal(t, ut):
                    slots[qb].append((j, ui))
                    break
            else:
                slots[qb].append((j, len(uniq)))
                uniq.append(t)
    return np.stack(uniq).astype(BF16), slots


_LT_UNIQ, _LT_SLOTS = _lt_tiles()
NU = _LT_UNIQ.shape[0]

_CACHE = {}


def _pin_act_tables(nc, mybir):
    """Make natural_log_exp_and_others the only set serving Exp/Ln so the
    table-load pass emits exactly one ACT_TABLE_LOAD (the baseline thrashed
    between exp_and_others and natural_log every softmax epilogue)."""
    from concourse.hw_specs import get_activation_tables
    AFT = mybir.ActivationFunctionType
    tabs = get_activation_tables(nc.m.arch)
    if "natural_log_exp_and_others" in tabs:
        both = tabs["natural_log_exp_and_others"]
        if AFT.Exp in both and AFT.Ln in both:
            for name, funcs in tabs.items():
                if name != "natural_log_exp_and_others":
                    funcs.discard(AFT.Exp)
                    funcs.discard(AFT.Ln)


def _build_program():
    import concourse.bacc as bacc
    import concourse.mybir as mybir
    from concourse.tile import TileContext

    f32 = mybir.dt.float32
    bf16 = mybir.dt.bfloat16
    f8 = mybir.dt.float8e4
    i16 = mybir.dt.int16
    Exp = mybir.ActivationFunctionType.Exp
    Ln = mybir.ActivationFunctionType.Ln
    mult = mybir.AluOpType.mult
    add = mybir.AluOpType.add
    DR = mybir.MatmulPerfMode.DoubleRow

    nc = bacc.Bacc("TRN2", target_bir_lowering=False, debug=False,
                   enable_asserts=False, num_devices=8)
    _pin_act_tables(nc, mybir)

    xt = nc.dram_tensor("xt", [D, S], bf16, kind="ExternalInput").ap()
    xtf = nc.dram_tensor("xtf", [D, S], f8, kind="ExternalInput").ap()
    wqt = nc.dram_tensor("wqt", [D, CLOC], f8, kind="ExternalInput").ap()
    wkt = nc.dram_tensor("wkt", [D, CLOC], f8, kind="ExternalInput").ap()
    wvt = nc.dram_tensor("wvt", [D, CLOC], bf16, kind="ExternalInput").ap()
    bqr = nc.dram_tensor("bqr", [1, CLOC], bf16, kind="ExternalInput").ap()
    bkr = nc.dram_tensor("bkr", [1, CLOC], bf16, kind="ExternalInput").ap()
    bvr = nc.dram_tensor("bvr", [1, CLOC], bf16, kind="ExternalInput").ap()
    wot = nc.dram_tensor("wot", [CLOC, D], bf16, kind="ExternalInput").ap()
    ltt = nc.dram_tensor("ltt", [NU, 128, QB], bf16, kind="ExternalInput").ap()
    out = nc.dram_tensor("out", [S, D], bf16, kind="ExternalOutput").ap()

    with TileContext(nc) as tc:
        with (
            tc.tile_pool(name="persist", bufs=1) as pp,
            tc.tile_pool(name="projsb", bufs=1) as prs,
            tc.tile_pool(name="stage", bufs=2) as sp,
            tc.tile_pool(name="pt", bufs=18) as ptp,
            tc.tile_pool(name="osb", bufs=4) as osb,
            tc.tile_pool(name="stps", bufs=2, space="PSUM") as stp,
            tc.tile_pool(name="ctxps", bufs=2, space="PSUM") as ctp,
            tc.tile_pool(name="auxps", bufs=2, space="PSUM") as axp,
        ):
            # ---- persistent SBUF ----
            qt = [pp.tile([128, S], bf16, name=f"qt{i}") for i in range(2)]
            kt = [pp.tile([128, S], bf16, name=f"kt{i}") for i in range(2)]
            v_all = pp.tile([128, NKC, NH, HD + 1], bf16)  # ones col at 64
            ctxt_all = pp.tile([128, 2, S], bf16)
            wot_sb = pp.tile([128, 2, D], bf16)
            bq_sb = pp.tile([1, CLOC], bf16)
            bk_sb = pp.tile([1, CLOC], bf16)
            bv_sb = pp.tile([1, CLOC], bf16)
            lt_sb = pp.tile([128, NU, QB], bf16)
            ones_r = pp.tile([1, QB], bf16)          # ones row (bias outer prod)
            ones_c = pp.tile([1, 128], bf16)         # ones row (V bias)
            xt_sb = prs.tile([128, NDC, S], bf16)
            xtf_sb = prs.tile([128, NDC, S], f8)
            wq_sb = prs.tile([128, NDC, CLOC], f8)
            wk_sb = prs.tile([128, NDC, CLOC], f8)
            wv_sb = prs.tile([128, NDC, CLOC], bf16)

            nc.gpsimd.memset(ones_r[:], 1.0)
            nc.gpsimd.memset(ones_c[:], 1.0)
            nc.gpsimd.memset(v_all[:, :, :, HD:HD + 1], 1.0)

            # Input DMA: each dma_start costs ~600ns of ISSUE time on its
            # engine's queue, so spread the issues across the three DMA-
            # capable queues (sync/scalar/gpsimd); fp8 operands first (the
            # ct0 projections start on them).
            for dc in range(NDC):
                if dc < 2:
                    # first chunk pair split so the ct0 projections start
                    # a few us earlier
                    for piece in range(2):
                        psl = slice(piece * 1024, (piece + 1) * 1024)
                        nc.sync.dma_start(xtf_sb[:, dc, psl],
                                          xtf[dc * 128:(dc + 1) * 128, psl])
                else:
                    nc.sync.dma_start(xtf_sb[:, dc, :],
                                      xtf[dc * 128:(dc + 1) * 128, :])
                nc.scalar.dma_start(wq_sb[:, dc, :],
                                    wqt[dc * 128:(dc + 1) * 128, :])
                nc.gpsimd.dma_start(wk_sb[:, dc, :],
                                    wkt[dc * 128:(dc + 1) * 128, :])
            for dc in range(NDC):
                for piece in range(2):
                    psl = slice(piece * 1024, (piece + 1) * 1024)
                    nc.sync.dma_start(xt_sb[:, dc, psl],
                                      xt[dc * 128:(dc + 1) * 128, psl])
            for dc in range(NDC):
                nc.scalar.dma_start(wv_sb[:, dc, :],
                                    wvt[dc * 128:(dc + 1) * 128, :])
            nc.scalar.dma_start(bq_sb[:], bqr[:])
            nc.scalar.dma_start(bk_sb[:], bkr[:])
            nc.scalar.dma_start(bv_sb[:], bvr[:])
            for cc in range(2):
                nc.gpsimd.dma_start(wot_sb[:, cc, :],
                                    wot[cc * 128:(cc + 1) * 128, :])
            for u in range(NU):
                nc.gpsimd.dma_start(lt_sb[:, u, :], ltt[u, :, :])

            # ---- phase B: c-tile-0 Q/K projections, dc-outer so matmuls
            # start as soon as the first xt/w chunks land ----
            for w_sb, b_sb, dst in ((wq_sb, bq_sb, qt[0]), (wk_sb, bk_sb, kt[0])):
                sts = [stp.tile([128, 2, QB], f32, tag="st", name=f"pj{h}")
                       for h in range(2)]
                for dcp in range(NDC // 2):
                    for qb in range(NQB):
                        # fp8 DoubleRow: contracts two 128-d chunks per pass
                        nc.tensor.matmul(
                            sts[qb // 2][:, qb % 2, :],
                            w_sb[:, 2 * dcp:2 * dcp + 2, 0:128],
                            xtf_sb[:, 2 * dcp:2 * dcp + 2,
                                   qb * QB:(qb + 1) * QB],
                            start=(dcp == 0), stop=False, perf_mode=DR)
                for qb in range(NQB):
                    nc.tensor.matmul(
                        sts[qb // 2][:, qb % 2, :], b_sb[:, 0:128], ones_r[:],
                        start=False, stop=True)
                    nc.scalar.copy(dst[:, qb * QB:(qb + 1) * QB],
                                   sts[qb // 2][:, qb % 2, :])

            # ---- V projection groups (fused into attention qb0: one s-chunk
            # per k-chunk slot, so exp starts right after the ct0 QK
            # projections while the PE fills with V matmuls) ----
            vstate = {}

            def v_group(sc):
                if sc % 2 == 0:
                    vstate["ps"] = axp.tile([128, QB], f32, tag="aux",
                                            name="vps")
                view = vstate["ps"][:, (sc % 2) * CLOC:(sc % 2 + 1) * CLOC]
                for dc in range(NDC):
                    nc.tensor.matmul(
                        view,
                        xt_sb[:, dc, sc * 128:(sc + 1) * 128],
                        wv_sb[:, dc, :],
                        start=(dc == 0), stop=False)
                nc.tensor.matmul(view, ones_c[:], bv_sb[:],
                                 start=False, stop=True)
                eng_copy = (nc.scalar.copy if sc % 2 else
                            nc.vector.tensor_copy)
                eng_copy(v_all[:, sc, :, 0:HD],
                         view.rearrange("p (h e) -> p h e", h=NH))

            # ---- injected work units (emitted interleaved into phase C) ----
            inject = []

            def ct1_group(w_sb, b_sb, dst, qb2):
                def emit():
                    ps = axp.tile([128, QB], f32, tag="aux")
                    for dcp in range(NDC // 2):
                        nc.tensor.matmul(
                            ps[:], w_sb[:, 2 * dcp:2 * dcp + 2, 128:256],
                            xtf_sb[:, 2 * dcp:2 * dcp + 2,
                                   qb2 * QB:(qb2 + 1) * QB],
                            start=(dcp == 0), stop=False, perf_mode=DR)
                    nc.tensor.matmul(ps[:], b_sb[:, 128:256], ones_r[:],
                                     start=False, stop=True)
                    nc.vector.tensor_copy(dst[:, qb2 * QB:(qb2 + 1) * QB], ps[:])
                return emit

            for qb2 in range(NQB):
                inject.append(ct1_group(wq_sb, bq_sb, qt[1], qb2))
                inject.append(ct1_group(wk_sb, bk_sb, kt[1], qb2))

            def d_unit(sc, ot):
                def emit():
                    ps = axp.tile([128, QB], f32, tag="aux")
                    for cc in range(2):
                        nc.tensor.matmul(
                            ps[:],
                            ctxt_all[:, cc, sc * 128:(sc + 1) * 128],
                            wot_sb[:, cc, ot * QB:(ot + 1) * QB],
                            start=(cc == 0), stop=(cc == 1))
                    ob = osb.tile([128, QB], bf16, tag="osb")
                    # spread casts and DMA issues across engine queues
                    if (sc + ot) % 2:
                        nc.scalar.copy(ob[:], ps[:])
                        nc.gpsimd.dma_start(
                            out[sc * 128:(sc + 1) * 128,
                                ot * QB:(ot + 1) * QB], ob[:])
                    else:
                        nc.vector.tensor_copy(ob[:], ps[:])
                        nc.sync.dma_start(
                            out[sc * 128:(sc + 1) * 128,
                                ot * QB:(ot + 1) * QB], ob[:])
                return emit

            def drain(n):
                for _ in range(n):
                    if inject:
                        inject.pop(0)()

            # ---- softmax epilogue (deferred emission) ----
            def epilogue(hp, qb, ctx, lv_ps, final=False):
                qsl = slice(qb * QB, (qb + 1) * QB)
                for hh in range(2):
                    # 1/rowsum = exp(-ln r) on ScalarE (same ACT table set)
                    lnr = sp.tile([65, QB], f32, tag="lnr")
                    nc.scalar.activation(lnr[64:65, :], ctx[hh][64:65, :], Ln)
                    bc_src = sp.tile([1, QB], f32, tag="bcsrc")
                    nc.scalar.activation(bc_src[0:1, :], lnr[64:65, :],
                                         Exp, scale=-1.0)
                    m1 = sp.tile([64, QB], f32, tag="m1")
                    bc_sb = sp.tile([64, QB], f32, tag="bcsb")
                    nc.gpsimd.partition_broadcast(bc_sb[:], bc_src[:],
                                                  channels=HD)
                    nc.vector.tensor_mul(m1[:], ctx[hh][0:HD, :], bc_sb[:])
                    # blend: (PV/r)*0.6 + 0.4LV, staged out per q-block
                    stg = sp.tile([64, QB], bf16, tag="stg")
                    nc.vector.scalar_tensor_tensor(
                        stg[:], m1[:], 0.6,
                        lv_ps[hh * HD:(hh + 1) * HD, :],
                        op0=mult, op1=add)
                    nc.sync.dma_start(
                        ctxt_all[hh * 64:hh * 64 + 64, hp, qsl], stg[:])

            # ---- phase C: attention per (head-pair, q-block) ----
            # kc loop is software-pipelined in EMISSION order: scores(kc) and
            # exp(kc) are issued before PV(kc-1), so the static per-engine
            # schedule never puts a PV (which waits on exp) ahead of the next
            # scores -- the exp stream stays saturated.
            pending = None     # deferred epilogue (hp, qb, ctx, lv)
            for hp in range(2):
                for qb in range(NQB):
                    # ct1 projections spread over hp0; D units 8/qb over
                    # hp1 (the last q-block keeps 2 for the tail window)
                    if hp == 0:
                        drain_kcs = (3, 7, 11)
                    elif qb < NQB - 1:
                        drain_kcs = (1, 3, 5, 7, 9, 11, 13, 15)
                    else:
                        drain_kcs = (1, 3, 5, 7, 9, 11, 13, 15)
                    fuse_v = (hp == 0 and qb == 0)
                    qsl = slice(qb * QB, (qb + 1) * QB)
                    ctx = None
                    pts = [None] * NKC
                    for kc in range(NKC):
                        st_ps = stp.tile([128, 2, QB], f32, tag="st")
                        for hh in range(2):
                            p0 = hh * 64
                            # K=64 row-group packing: the head pair runs
                            # concurrently on disjoint PE row groups
                            nc.tensor.matmul(
                                st_ps[:, hh, :],
                                kt[hp][p0:p0 + 64, kc * 128:(kc + 1) * 128],
                                qt[hp][p0:p0 + 64, qsl],
                                start=True, stop=True,
                                tile_position=(p0, 0))
                        pt_sb = ptp.tile([128, 2, QB], bf16, tag="pt")
                        pts[kc] = pt_sb
                        if kc in DVE_KCS:
                            # int16 bit-trick exp on the DVE (bitcast bf16)
                            nc.vector.tensor_scalar(
                                pt_sb[:].bitcast(i16), st_ps[:],
                                A_EXP * SCORE_SCALE, B_EXP,
                                op0=mult, op1=add)
                        else:
                            nc.scalar.activation(pt_sb[:], st_ps[:], Exp,
                                                 scale=SCORE_SCALE)
                        if fuse_v:
                            # V projection rides the qb0 exp stream; PV for
                            # qb0 runs as a burst after V completes
                            v_group(kc)
                            continue
                        if kc == 1:
                            # flush the previous q-block's epilogue AFTER
                            # this block's first scores+exp (keeps ScalarE
                            # fed) but BEFORE the ctx pool rotates
                            if pending is not None:
                                php, pqb, pctx, plv = pending
                                epilogue(php, pqb, pctx, plv)
                                pending = None
                                if php == 1:
                                    # hp1 epilogue flushed: its out-proj
                                    # tiles become available to inject
                                    for sc in range(4 * pqb, 4 * pqb + 4):
                                        for ot in range(2):
                                            inject.append(d_unit(sc, ot))
                            ctx = [ctp.tile([128, QB], f32, tag="ctxps",
                                            name=f"ctx{h}") for h in range(2)]
                        if kc >= 1:
                            for hh in range(2):
                                nc.tensor.matmul(
                                    ctx[hh][0:HD + 1, :],
                                    v_all[:, kc - 1, 2 * hp + hh, 0:HD + 1],
                                    pts[kc - 1][:, hh, :],
                                    start=(kc == 1), stop=False)
                        if kc in drain_kcs:
                            drain(1)
                    if fuse_v:
                        ctx = [ctp.tile([128, QB], f32, tag="ctxps",
                                        name=f"ctx{h}") for h in range(2)]
                        for kc in range(NKC - 1):
                            for hh in range(2):
                                nc.tensor.matmul(
                                    ctx[hh][0:HD + 1, :],
                                    v_all[:, kc, 2 * hp + hh, 0:HD + 1],
                                    pts[kc][:, hh, :],
                                    start=(kc == 0), stop=False)
                    for hh in range(2):
                        nc.tensor.matmul(
                            ctx[hh][0:HD + 1, :],
                            v_all[:, NKC - 1, 2 * hp + hh, 0:HD + 1],
                            pts[NKC - 1][:, hh, :],
                            start=False, stop=True)
                    # banded 0.4*L^T term, both heads column-packed into
                    # one psum tile (col strips 0-1 / 2-3 run concurrently)
                    slots = _LT_SLOTS[qb]
                    lv_ps = axp.tile([128, QB], f32, tag="aux", name="lv")
                    for n, (j, u) in enumerate(slots):
                        for hh in range(2):
                            nc.tensor.matmul(
                                lv_ps[hh * HD:(hh + 1) * HD, :],
                                v_all[:, j, 2 * hp + hh, 0:HD],
                                lt_sb[:, u, :],
                                start=(n == 0), stop=(n == len(slots) - 1),
                                tile_position=(0, hh * HD),
                                skip_group_check=True)
                    pending = (hp, qb, ctx, lv_ps)
            # ---- tail: last q-block's epilogue + out-projection ----
            # PE-side broadcast keeps the PE busy through the serial chain
            # (a >3.4us PE idle here re-throttles HAM and the tail matmuls
            # would run at half clock); D psums ride the freed scores pool.
            epilogue(pending[0], pending[1], pending[2], pending[3],
                     final=True)
            drain(len(inject))          # 4 reserved units fill the chain
            dtile = [stp.tile([128, 2, QB], f32, tag="st", name=f"dt{i}")
                     for i in range(2)]
            for u, (sc, ot) in enumerate(
                    (sc, ot) for sc in range(4 * (NQB - 1), 4 * NQB)
                    for ot in range(2)):
                if u < 4:
                    ps = dtile[u // 2][:, u % 2, :]
                else:
                    ps = axp.tile([128, QB], f32, tag="aux",
                                  name=f"dtail{u}")[:]
                for cc in range(2):
                    nc.tensor.matmul(
                        ps,
                        ctxt_all[:, cc, sc * 128:(sc + 1) * 128],
                        wot_sb[:, cc, ot * QB:(ot + 1) * QB],
                        start=(cc == 0), stop=(cc == 1))
                ob = osb.tile([128, QB], bf16, tag="osb")
                eng_copy = nc.scalar.copy if u % 2 else nc.vector.tensor_copy
                eng_copy(ob[:], ps)
                # split the last output DMAs fine so the drain overlaps
                for half, eng in ((0, nc.sync), (1, nc.gpsimd)):
                    hsl = slice(half * 256, (half + 1) * 256)
                    eng.dma_start(
                        out[sc * 128:(sc + 1) * 128, ot * QB + half * 256:
                            ot * QB + half * 256 + 256],
                        ob[:, hsl])

    nc.compile()
    return nc


def _get_program():
    if "nc" not in _CACHE:
        _CACHE["nc"] = _build_program()
    return _CACHE["nc"]


def _in_maps(x, Wq, bq, Wk, bk, Wv, bv, Wo):
    xT = [np.ascontiguousarray(x[b].T).astype(BF16) for b in range(2)]
    maps = []
    for c in range(8):
        b, hg = c // 4, c % 4
        hs, he = hg * CLOC, (hg + 1) * CLOC
        maps.append({
            "xt": xT[b],
            "xtf": xT[b].astype(F8),
            "wqt": np.ascontiguousarray(
                Wq[hs:he].T * F32(SW / 8.0)).astype(F8),
            "wkt": np.ascontiguousarray(Wk[hs:he].T * F32(SW)).astype(F8),
            "wvt": np.ascontiguousarray(Wv[hs:he].T).astype(BF16),
            "bqr": (bq[hs:he] * F32(SW / 8.0))[None, :].astype(BF16),
            "bkr": (bk[hs:he] * F32(SW))[None, :].astype(BF16),
            "bvr": bv[hs:he][None, :].astype(BF16),
            "wot": np.ascontiguousarray(Wo[:, hs:he].T).astype(BF16),
            "ltt": _LT_UNIQ,
        })
    return maps


def _run(x, Wq, bq, Wk, bk, Wv, bv, Wo, bo, trace=False):
    from concourse.bass_utils import run_bass_kernel_spmd
    nc = _get_program()
    maps = _in_maps(np.asarray(x, F32), np.asarray(Wq, F32), np.asarray(bq, F32),
                    np.asarray(Wk, F32), np.asarray(bk, F32), np.asarray(Wv, F32),
                    np.asarray(bv, F32), np.asarray(Wo, F32))
    res = run_bass_kernel_spmd(nc, maps, list(range(8)), trace=trace)
    bo = np.asarray(bo, F32)
    outp = np.empty((2, S, D), F32)
    for b in range(2):
        acc = res.results[b * 4]["out"].astype(F32)
        for hg in range(1, 4):
            acc = acc + res.results[b * 4 + hg]["out"].astype(F32)
        outp[b] = acc + bo
    return outp, res


def kernel(x, Wq, bq, Wk, bk, Wv, bv, Wo, bo):
    outp, _ = _run(x, Wq, bq, Wk, bk, Wv, bv, Wo, bo, trace=False)
    return outp


def kernel_traced(**inputs):
    return _run(trace=True, **inputs)


# revision 45
# speedup vs baseline: 1.0022x; 1.0022x over previous
"""Trainium2 Bass kernel for DampedAttention.

Full inputs in, full output out. Sharding: 8 cores = 2 batches x 4 head-groups
(4 heads of dim 64 each per core). Per core:

  QT/KT  [c, s] transposed projections (c on partitions), scale 1/8 and biases
         folded in (bias via K=1 ones-row matmuls, scale into weights on host)
  V      [s, c] natural projection (lhsT for the P@V matmul)
  ST     scores transposed [k, q] per (k-chunk, q-block) so exp(ST) is directly
         the lhsT-layout P^T needed by P@V -- no on-chip transposes
  ctxT   [65, q] = V_aug^T @ P^T ; row 64 = softmax row-sums (ones column in V)
  LVT    [64, q] banded 0.4*L^T matmuls (8 unique host-built band tiles)
  blend  ctxT_final = PV * (0.6/r, bcast over partitions) + LVT
  out    [s, o] natural out-projection; host sums 4 head-group partials + bo

v2 restructure (vs the 315us baseline):
  - exp/ln pinned to the natural_log_exp_and_others ACT table set (the
    baseline thrashed 17 ACT_TABLE_LOADs between exp_and_others/natural_log)
  - projections emitted dc-outer so the PE starts as soon as the first xt
    chunk lands (baseline idled 25us waiting for the full input DMA)
  - c-tile-1 Q/K projections are emitted interleaved into attention hp=0,
    and the out-projection per q-block interleaved into hp=1, so ScalarE
    (the critical engine) starts exp ~30us in and never drains
  - softmax epilogue emission deferred past the next q-block's first
    scores+exp so the ScalarE stream has no per-q-block bubble
  - 5 of 16 k-chunks compute exp on the DVE via the int16 bit-trick
    (bitcast bf16 exp), offloading the ScalarE bottleneck
  - output partials in bf16 (halves the output DMA)

Matmul operands are bf16; accumulation, softmax row-sums, reciprocal and the
0.6/r normalization stay fp32. The entropy gate in the reference is a forward
no-op and is skipped. Softmax max-subtraction is skipped (scores are O(1)).
"""
import numpy as np
import ml_dtypes

S = 2048
D = 1024
CLOC = 256          # channels per core (4 heads x 64)
HD = 64
NH = 4              # heads per core
NDC = 8             # 128-wide d-chunks in contraction D
NKC = 16            # 128-wide k/s chunks in S
NQB = 4             # 512-wide q blocks
QB = 512
WINDOW = 3
STRENGTH = 0.4
EPS = 1e-10
F32 = np.float32
BF16 = ml_dtypes.bfloat16
F8 = ml_dtypes.float8_e4m3
SW = 256.0   # fp8 weight pre-scale (keeps W out of fp8 subnormals);
             # 1/SW^2 is folded into the exp() input scale

# DVE bit-trick exp: exp(x) ~= bitcast_bf16(int16(x * 128/ln2 + 16256 - C))
# C tunes the sawtooth bias; +0.5 emulates round-to-nearest under truncation.
A_EXP = float(128.0 / np.log(2.0))
B_EXP = float(16256 - 8 + 0.5)
# alternate exp between DVE (evens) and ScalarE (odds): consecutive chunks'
# exps run on different engines concurrently, so the scores-psum recycling
# chain (bufs=2) never clocks the attention loop at one engine's exp latency
DVE_KCS = (0, 2, 4, 6, 8, 10, 12, 14)
SCORE_SCALE = 1.0 / (SW * SW)   # undo the fp8 weight pre-scale at exp time


def _build_L04T():
    i = np.arange(S)
    d = (i[:, None] - i[None, :]).astype(F32)
    k = np.where(np.abs(d) <= WINDOW,
                 np.exp(-(d ** 2) / F32(2.0 * STRENGTH ** 2)),
                 F32(0.0)).astype(F32)
    L = k / (k.sum(axis=-1, keepdims=True) + F32(EPS))
    return (F32(0.4) * L).T.copy()  # [s, q], pre-scaled by (1 - lambda_jump)


def _lt_tiles():
    """Unique [128, 512] band tiles of 0.4*L^T plus (qb -> [(j, uniq_idx)])."""
    L04T = _build_L04T()
    uniq = []
    slots = {qb: [] for qb in range(NQB)}
    for qb in range(NQB):
        for j in range(max(0, qb * 4 - 1), min(NKC, qb * 4 + 5)):
            t = L04T[j * 128:(j + 1) * 128, qb * QB:(qb + 1) * QB]
            for ui, ut in enumerate(uniq):
                if np.array_equal(t, ut):
                    slots[qb].append((j, ui))
                    break
            else:
                slots[qb].append((j, len(uniq)))
                uniq.append(t)
    return np.stack(uniq).astype(BF16), slots


_LT_UNIQ, _LT_SLOTS = _lt_tiles()
NU = _LT_UNIQ.shape[0]

_CACHE = {}


def _pin_act_tables(nc, mybir):
    """Make natural_log_exp_and_others the only set serving Exp/Ln so the
    table-load pass emits exactly one ACT_TABLE_LOAD (the baseline thrashed
    between exp_and_others and natural_log every softmax epilogue)."""
    from concourse.hw_specs import get_activation_tables
    AFT = mybir.ActivationFunctionType
    tabs = get_activation_tables(nc.m.arch)
    if "natural_log_exp_and_others" in tabs:
        both = tabs["natural_log_exp_and_others"]
        if AFT.Exp in both and AFT.Ln in both:
            for name, funcs in tabs.items():
                if name != "natural_log_exp_and_others":
                    funcs.discard(AFT.Exp)
                    funcs.discard(AFT.Ln)


def _build_program():
    import concourse.bacc as bacc
    import concourse.mybir as mybir
    from concourse.tile import TileContext

    f32 = mybir.dt.float32
    bf16 = mybir.dt.bfloat16
    f8 = mybir.dt.float8e4
    i16 = mybir.dt.int16
    Exp = mybir.ActivationFunctionType.Exp
    Ln = mybir.ActivationFunctionType.Ln
    mult = mybir.AluOpType.mult
    add = mybir.AluOpType.add
    DR = mybir.MatmulPerfMode.DoubleRow

    nc = bacc.Bacc("TRN2", target_bir_lowering=False, debug=False,
                   enable_asserts=False, num_devices=8)
    _pin_act_tables(nc, mybir)

    xt = nc.dram_tensor("xt", [D, S], bf16, kind="ExternalInput").ap()
    xtf = nc.dram_tensor("xtf", [D, S], f8, kind="ExternalInput").ap()
    wqt = nc.dram_tensor("wqt", [D, CLOC], f8, kind="ExternalInput").ap()
    wkt = nc.dram_tensor("wkt", [D, CLOC], f8, kind="ExternalInput").ap()
    wvt = nc.dram_tensor("wvt", [D, CLOC], bf16, kind="ExternalInput").ap()
    bqr = nc.dram_tensor("bqr", [1, CLOC], bf16, kind="ExternalInput").ap()
    bkr = nc.dram_tensor("bkr", [1, CLOC], bf16, kind="ExternalInput").ap()
    bvr = nc.dram_tensor("bvr", [1, CLOC], bf16, kind="ExternalInput").ap()
    wot = nc.dram_tensor("wot", [CLOC, D], bf16, kind="ExternalInput").ap()
    ltt = nc.dram_tensor("ltt", [NU, 128, QB], bf16, kind="ExternalInput").ap()
    out = nc.dram_tensor("out", [S, D], bf16, kind="ExternalOutput").ap()

    with TileContext(nc) as tc:
        with (
            tc.tile_pool(name="persist", bufs=1) as pp,
            tc.tile_pool(name="projsb", bufs=1) as prs,
            tc.tile_pool(name="stage", bufs=2) as sp,
            tc.tile_pool(name="pt", bufs=18) as ptp,
            tc.tile_pool(name="osb", bufs=4) as osb,
            tc.tile_pool(name="stps", bufs=2, space="PSUM") as stp,
            tc.tile_pool(name="ctxps", bufs=2, space="PSUM") as ctp,
            tc.tile_pool(name="auxps", bufs=2, space="PSUM") as axp,
        ):
            # ---- persistent SBUF ----
            qt = [pp.tile([128, S], bf16, name=f"qt{i}") for i in range(2)]
            kt = [pp.tile([128, S], bf16, name=f"kt{i}") for i in range(2)]
            v_all = pp.tile([128, NKC, NH, HD + 1], bf16)  # ones col at 64
            ctxt_all = pp.tile([128, 2, S], bf16)
            wot_sb = pp.tile([128, 2, D], bf16)
            bq_sb = pp.tile([1, CLOC], bf16)
            bk_sb = pp.tile([1, CLOC], bf16)
            bv_sb = pp.tile([1, CLOC], bf16)
            lt_sb = pp.tile([128, NU, QB], bf16)
            ones_r = pp.tile([1, QB], bf16)          # ones row (bias outer prod)
            ones_c = pp.tile([1, 128], bf16)         # ones row (V bias)
            xt_sb = prs.tile([128, NDC, S], bf16)
            xtf_sb = prs.tile([128, NDC, S], f8)
            wq_sb = prs.tile([128, NDC, CLOC], f8)
            wk_sb = prs.tile([128, NDC, CLOC], f8)
            wv_sb = prs.tile([128, NDC, CLOC], bf16)

            nc.gpsimd.memset(ones_r[:], 1.0)
            nc.gpsimd.memset(ones_c[:], 1.0)
            nc.gpsimd.memset(v_all[:, :, :, HD:HD + 1], 1.0)

            # Input DMA: each dma_start costs ~600ns of ISSUE time on its
            # engine's queue, so spread the issues across the three DMA-
            # capable queues (sync/scalar/gpsimd); fp8 operands first (the
            # ct0 projections start on them).
            for dc in range(NDC):
                nc.sync.dma_start(xtf_sb[:, dc, :],
                                  xtf[dc * 128:(dc + 1) * 128, :])
                nc.scalar.dma_start(wq_sb[:, dc, :],
                                    wqt[dc * 128:(dc + 1) * 128, :])
                nc.scalar.dma_start(wk_sb[:, dc, :],
                                    wkt[dc * 128:(dc + 1) * 128, :])
            for dc in range(NDC):
                for piece in range(2):
                    psl = slice(piece * 1024, (piece + 1) * 1024)
                    nc.sync.dma_start(xt_sb[:, dc, psl],
                                      xt[dc * 128:(dc + 1) * 128, psl])
            for dc in range(NDC):
                nc.gpsimd.dma_start(wv_sb[:, dc, :],
                                    wvt[dc * 128:(dc + 1) * 128, :])
            nc.gpsimd.dma_start(bq_sb[:], bqr[:])
            nc.gpsimd.dma_start(bk_sb[:], bkr[:])
            nc.gpsimd.dma_start(bv_sb[:], bvr[:])
            for cc in range(2):
                nc.gpsimd.dma_start(wot_sb[:, cc, :],
                                    wot[cc * 128:(cc + 1) * 128, :])
            for u in range(NU):
                nc.gpsimd.dma_start(lt_sb[:, u, :], ltt[u, :, :])

            # ---- phase B: c-tile-0 Q/K projections, dc-outer so matmuls
            # start as soon as the first xt/w chunks land ----
            for w_sb, b_sb, dst in ((wq_sb, bq_sb, qt[0]), (wk_sb, bk_sb, kt[0])):
                sts = [stp.tile([128, 2, QB], f32, tag="st", name=f"pj{h}")
                       for h in range(2)]
                for dcp in range(NDC // 2):
                    for qb in range(NQB):
                        # fp8 DoubleRow: contracts two 128-d chunks per pass
                        nc.tensor.matmul(
                            sts[qb // 2][:, qb % 2, :],
                            w_sb[:, 2 * dcp:2 * dcp + 2, 0:128],
                            xtf_sb[:, 2 * dcp:2 * dcp + 2,
                                   qb * QB:(qb + 1) * QB],
                            start=(dcp == 0), stop=False, perf_mode=DR)
                for qb in range(NQB):
                    nc.tensor.matmul(
                        sts[qb // 2][:, qb % 2, :], b_sb[:, 0:128], ones_r[:],
                        start=False, stop=True)
                    nc.scalar.copy(dst[:, qb * QB:(qb + 1) * QB],
                                   sts[qb // 2][:, qb % 2, :])

            # ---- V projection groups (fused into attention qb0: one s-chunk
            # per k-chunk slot, so exp starts right after the ct0 QK
            # projections while the PE fills with V matmuls) ----
            vstate = {}

            def v_group(sc):
                if sc % 2 == 0:
                    vstate["ps"] = axp.tile([128, QB], f32, tag="aux",
                                            name="vps")
                view = vstate["ps"][:, (sc % 2) * CLOC:(sc % 2 + 1) * CLOC]
                for dc in range(NDC):
                    nc.tensor.matmul(
                        view,
                        xt_sb[:, dc, sc * 128:(sc + 1) * 128],
                        wv_sb[:, dc, :],
                        start=(dc == 0), stop=False)
                nc.tensor.matmul(view, ones_c[:], bv_sb[:],
                                 start=False, stop=True)
                eng_copy = (nc.scalar.copy if sc % 2 else
                            nc.vector.tensor_copy)
                eng_copy(v_all[:, sc, :, 0:HD],
                         view.rearrange("p (h e) -> p h e", h=NH))

            # ---- injected work units (emitted interleaved into phase C) ----
            inject = []

            def ct1_group(w_sb, b_sb, dst, qb2):
                def emit():
                    ps = axp.tile([128, QB], f32, tag="aux")
                    for dcp in range(NDC // 2):
                        nc.tensor.matmul(
                            ps[:], w_sb[:, 2 * dcp:2 * dcp + 2, 128:256],
                            xtf_sb[:, 2 * dcp:2 * dcp + 2,
                                   qb2 * QB:(qb2 + 1) * QB],
                            start=(dcp == 0), stop=False, perf_mode=DR)
                    nc.tensor.matmul(ps[:], b_sb[:, 128:256], ones_r[:],
                                     start=False, stop=True)
                    nc.vector.tensor_copy(dst[:, qb2 * QB:(qb2 + 1) * QB], ps[:])
                return emit

            for qb2 in range(NQB):
                inject.append(ct1_group(wq_sb, bq_sb, qt[1], qb2))
                inject.append(ct1_group(wk_sb, bk_sb, kt[1], qb2))

            def d_unit(sc, ot):
                def emit():
                    ps = axp.tile([128, QB], f32, tag="aux")
                    for cc in range(2):
                        nc.tensor.matmul(
                            ps[:],
                            ctxt_all[:, cc, sc * 128:(sc + 1) * 128],
                            wot_sb[:, cc, ot * QB:(ot + 1) * QB],
                            start=(cc == 0), stop=(cc == 1))
                    ob = osb.tile([128, QB], bf16, tag="osb")
                    # spread casts and DMA issues across engine queues
                    if (sc + ot) % 2:
                        nc.scalar.copy(ob[:], ps[:])
                        nc.gpsimd.dma_start(
                            out[sc * 128:(sc + 1) * 128,
                                ot * QB:(ot + 1) * QB], ob[:])
                    else:
                        nc.vector.tensor_copy(ob[:], ps[:])
                        nc.sync.dma_start(
                            out[sc * 128:(sc + 1) * 128,
                                ot * QB:(ot + 1) * QB], ob[:])
                return emit

            def drain(n):
                for _ in range(n):
                    if inject:
                        inject.pop(0)()

            # ---- softmax epilogue (deferred emission) ----
            def epilogue(hp, qb, ctx, lv_ps, final=False):
                qsl = slice(qb * QB, (qb + 1) * QB)
                for hh in range(2):
                    # 1/rowsum = exp(-ln r) on ScalarE (same ACT table set)
                    lnr = sp.tile([65, QB], f32, tag="lnr")
                    nc.scalar.activation(lnr[64:65, :], ctx[hh][64:65, :], Ln)
                    bc_src = sp.tile([1, QB], f32, tag="bcsrc")
                    nc.scalar.activation(bc_src[0:1, :], lnr[64:65, :],
                                         Exp, scale=-1.0)
                    m1 = sp.tile([64, QB], f32, tag="m1")
                    bc_sb = sp.tile([64, QB], f32, tag="bcsb")
                    nc.gpsimd.partition_broadcast(bc_sb[:], bc_src[:],
                                                  channels=HD)
                    nc.vector.tensor_mul(m1[:], ctx[hh][0:HD, :], bc_sb[:])
                    # blend: (PV/r)*0.6 + 0.4LV, staged out per q-block
                    stg = sp.tile([64, QB], bf16, tag="stg")
                    nc.vector.scalar_tensor_tensor(
                        stg[:], m1[:], 0.6,
                        lv_ps[hh * HD:(hh + 1) * HD, :],
                        op0=mult, op1=add)
                    nc.sync.dma_start(
                        ctxt_all[hh * 64:hh * 64 + 64, hp, qsl], stg[:])

            # ---- phase C: attention per (head-pair, q-block) ----
            # kc loop is software-pipelined in EMISSION order: scores(kc) and
            # exp(kc) are issued before PV(kc-1), so the static per-engine
            # schedule never puts a PV (which waits on exp) ahead of the next
            # scores -- the exp stream stays saturated.
            pending = None     # deferred epilogue (hp, qb, ctx, lv)
            for hp in range(2):
                for qb in range(NQB):
                    # ct1 projections spread over hp0; D units 8/qb over
                    # hp1 (the last q-block keeps 2 for the tail window)
                    if hp == 0:
                        drain_kcs = (3, 7, 11)
                    elif qb < NQB - 1:
                        drain_kcs = (1, 3, 5, 7, 9, 11, 13, 15)
                    else:
                        drain_kcs = (1, 3, 5, 7, 9, 11, 13, 15)
                    fuse_v = (hp == 0 and qb == 0)
                    qsl = slice(qb * QB, (qb + 1) * QB)
                    ctx = None
                    pts = [None] * NKC
                    for kc in range(NKC):
                        st_ps = stp.tile([128, 2, QB], f32, tag="st")
                        for hh in range(2):
                            p0 = hh * 64
                            # K=64 row-group packing: the head pair runs
                            # concurrently on disjoint PE row groups
                            nc.tensor.matmul(
                                st_ps[:, hh, :],
                                kt[hp][p0:p0 + 64, kc * 128:(kc + 1) * 128],
                                qt[hp][p0:p0 + 64, qsl],
                                start=True, stop=True,
                                tile_position=(p0, 0))
                        pt_sb = ptp.tile([128, 2, QB], bf16, tag="pt")
                        pts[kc] = pt_sb
                        if kc in DVE_KCS:
                            # int16 bit-trick exp on the DVE (bitcast bf16)
                            nc.vector.tensor_scalar(
                                pt_sb[:].bitcast(i16), st_ps[:],
                                A_EXP * SCORE_SCALE, B_EXP,
                                op0=mult, op1=add)
                        else:
                            nc.scalar.activation(pt_sb[:], st_ps[:], Exp,
                                                 scale=SCORE_SCALE)
                        if fuse_v:
                            # V projection rides the qb0 exp stream; PV for
                            # qb0 runs as a burst after V completes
                            v_group(kc)
                            continue
                        if kc == 1:
                            # flush the previous q-block's epilogue AFTER
                            # this block's first scores+exp (keeps ScalarE
                            # fed) but BEFORE the ctx pool rotates
                            if pending is not None:
                                php, pqb, pctx, plv = pending
                                epilogue(php, pqb, pctx, plv)
                                pending = None
                                if php == 1:
                                    # hp1 epilogue flushed: its out-proj
                                    # tiles become available to inject
                                    for sc in range(4 * pqb, 4 * pqb + 4):
                                        for ot in range(2):
                                            inject.append(d_unit(sc, ot))
                            ctx = [ctp.tile([128, QB], f32, tag="ctxps",
                                            name=f"ctx{h}") for h in range(2)]
                        if kc >= 1:
                            for hh in range(2):
                                nc.tensor.matmul(
                                    ctx[hh][0:HD + 1, :],
                                    v_all[:, kc - 1, 2 * hp + hh, 0:HD + 1],
                                    pts[kc - 1][:, hh, :],
                                    start=(kc == 1), stop=False)
                        if kc in drain_kcs:
                            drain(1)
                    if fuse_v:
                        ctx = [ctp.tile([128, QB], f32, tag="ctxps",
                                        name=f"ctx{h}") for h in range(2)]
                        for kc in range(NKC - 1):
                            for hh in range(2):
                                nc.tensor.matmul(
                                    ctx[hh][0:HD + 1, :],
                                    v_all[:, kc, 2 * hp + hh, 0:HD + 1],
                                    pts[kc][:, hh, :],
                                    start=(kc == 0), stop=False)
                    for hh in range(2):
                        nc.tensor.matmul(
                            ctx[hh][0:HD + 1, :],
                            v_all[:, NKC - 1, 2 * hp + hh, 0:HD + 1],
                            pts[NKC - 1][:, hh, :],
                            start=False, stop=True)
                    # banded 0.4*L^T term, both heads column-packed into
                    # one psum tile (col strips 0-1 / 2-3 run concurrently)
                    slots = _LT_SLOTS[qb]
                    lv_ps = axp.tile([128, QB], f32, tag="aux", name="lv")
                    for n, (j, u) in enumerate(slots):
                        for hh in range(2):
                            nc.tensor.matmul(
                                lv_ps[hh * HD:(hh + 1) * HD, :],
                                v_all[:, j, 2 * hp + hh, 0:HD],
                                lt_sb[:, u, :],
                                start=(n == 0), stop=(n == len(slots) - 1),
                                tile_position=(0, hh * HD),
                                skip_group_check=True)
                    pending = (hp, qb, ctx, lv_ps)
            # ---- tail: last q-block's epilogue + out-projection ----
            # PE-side broadcast keeps the PE busy through the serial chain
            # (a >3.4us PE idle here re-throttles HAM and the tail matmuls
            # would run at half clock); D psums ride the freed scores pool.
            epilogue(pending[0], pending[1], pending[2], pending[3],
                     final=True)
            drain(len(inject))          # 4 reserved units fill the chain
            dtile = [stp.tile([128, 2, QB], f32, tag="st", name=f"dt{i}")
                     for i in range(2)]
            for u, (sc, ot) in enumerate(
                    (sc, ot) for sc in range(4 * (NQB - 1), 4 * NQB)
                    for ot in range(2)):
                if u < 4:
                    ps = dtile[u // 2][:, u % 2, :]
                else:
                    ps = axp.tile([128, QB], f32, tag="aux",
                                  name=f"dtail{u}")[:]
                for cc in range(2):
                    nc.tensor.matmul(
                        ps,
                        ctxt_all[:, cc, sc * 128:(sc + 1) * 128],
                        wot_sb[:, cc, ot * QB:(ot + 1) * QB],
                        start=(cc == 0), stop=(cc == 1))
                ob = osb.tile([128, QB], bf16, tag="osb")
                eng_copy = nc.scalar.copy if u % 2 else nc.vector.tensor_copy
                eng_copy(ob[:], ps)
                # split the last output DMAs fine so the drain overlaps
                for half, eng in ((0, nc.sync), (1, nc.gpsimd)):
                    hsl = slice(half * 256, (half + 1) * 256)
                    eng.dma_start(
                        out[sc * 128:(sc + 1) * 128, ot * QB + half * 256:
                            ot * QB + half * 256 + 256],
                        ob[:, hsl])

    nc.compile()
    return nc


def _get_program():
    if "nc" not in _CACHE:
        _CACHE["nc"] = _build_program()
    return _CACHE["nc"]


def _in_maps(x, Wq, bq, Wk, bk, Wv, bv, Wo):
    xT = [np.ascontiguousarray(x[b].T).astype(BF16) for b in range(2)]
    maps = []
    for c in range(8):
        b, hg = c // 4, c % 4
        hs, he = hg * CLOC, (hg + 1) * CLOC
        maps.append({
            "xt": xT[b],
            "xtf": xT[b].astype(F8),
            "wqt": np.ascontiguousarray(
                Wq[hs:he].T * F32(SW / 8.0)).astype(F8),
            "wkt": np.ascontiguousarray(Wk[hs:he].T * F32(SW)).astype(F8),
            "wvt": np.ascontiguousarray(Wv[hs:he].T).astype(BF16),
            "bqr": (bq[hs:he] * F32(SW / 8.0))[None, :].astype(BF16),
            "bkr": (bk[hs:he] * F32(SW))[None, :].astype(BF16),
            "bvr": bv[hs:he][None, :].astype(BF16),
            "wot": np.ascontiguousarray(Wo[:, hs:he].T).astype(BF16),
            "ltt": _LT_UNIQ,
        })
    return maps


def _run(x, Wq, bq, Wk, bk, Wv, bv, Wo, bo, trace=False):
    from concourse.bass_utils import run_bass_kernel_spmd
    nc = _get_program()
    maps = _in_maps(np.asarray(x, F32), np.asarray(Wq, F32), np.asarray(bq, F32),
                    np.asarray(Wk, F32), np.asarray(bk, F32), np.asarray(Wv, F32),
                    np.asarray(bv, F32), np.asarray(Wo, F32))
    res = run_bass_kernel_spmd(nc, maps, list(range(8)), trace=trace)
    bo = np.asarray(bo, F32)
    outp = np.empty((2, S, D), F32)
    for b in range(2):
        acc = res.results[b * 4]["out"].astype(F32)
        for hg in range(1, 4):
            acc = acc + res.results[b * 4 + hg]["out"].astype(F32)
        outp[b] = acc + bo
    return outp, res


def kernel(x, Wq, bq, Wk, bk, Wv, bv, Wo, bo):
    outp, _ = _run(x, Wq, bq, Wk, bk, Wv, bv, Wo, bo, trace=False)
    return outp


def kernel_traced(**inputs):
    return _run(trace=True, **inputs)
